# revision 29
# baseline (speedup 1.0000x reference)
"""CrossAttention Trainium2 kernel (8 NeuronCores, SPMD, no collectives).

Shapes: B=4, LQ=1024, LK=2048, QD=768, KD=VD=512, H=1024, NH=16, HD=64.

Sharding: core c = (b = c//2, head-half hh = c%2). Each core computes, for
its batch b, the 8 heads in H-column block [512*hh, 512*hh+512): the
q/k/v projections for that block, attention over all 1024 queries x 2048
keys, and the out-projection PARTIAL  att_half @ Wo[512*hh:+512, :].  The
host sums the two partials per batch (+bo).  No k/v-proj duplication.

Device-side dataflow per core (bf16 matmul operands, fp32 PSUM accum):
  qp = Wq_h.T @ q.T + bq          [512, 1024]   laid out [128, mt, 1024]
  kp = Wk_h.T @ k.T + bk          [512, 2048]   laid out [128, mt, 2048]
  vp = v @ Wv_h + bv              [2048, 512]   per-key-chunk [128, 8, 65]
       with a ones column per head (fused softmax-denominator row)
  per head h, key-chunk kc (128 keys), q-half qb (512 q):
     S^T[kc, qb] = kp_h_chunk.T @ qp_h_qb       (K=64 contraction)
     es = exp(S^T * 0.125)       (Scalar PSUM->SBUF bf16; 16.8M elem/core
                                  = the pacing engine at ~1.1us/tile)
     att^T[0:65, qb] += [v_chunk_h | 1].T @ es  (row 64 = denominator)
  normalize: rec = approx_recip(att^T[64]); gpsimd partition_broadcast;
             attT_h = att^T[0:64] * rec
  out_partial[qt] = attT[:, qt].T @ Wo_half     (K=512 contraction)

DMA: every large load is split into 4 transfers so its descriptors spread
across queues, and issue order follows need order (qt/wq -> kt/wk -> vt/
wv -> wo) so the first score matmul lands as early as possible.

Softmax skips max-subtraction: |scores/8| < ~2 by construction of the
problem's input scale, so exp is stable in fp32.
"""
import sys

if "/opt/trn_rl_repo" not in sys.path:
    sys.path.insert(0, "/opt/trn_rl_repo")

import numpy as np
import ml_dtypes

B, LQ, LK = 4, 1024, 2048
QD, KD, VD = 768, 512, 512
H, NH = 1024, 16
HD = H // NH          # 64
HH = H // 2           # 512 H-columns per core
NHH = NH // 2         # 8 heads per core
NCORES = 8

_BF = ml_dtypes.bfloat16
_NC_CACHE = {}


def build_nc(debug=False):
    import concourse.bacc as bacc
    import concourse.tile as tile
    from concourse import mybir

    f32 = mybir.dt.float32
    bf16 = mybir.dt.bfloat16
    AF = mybir.ActivationFunctionType

    nc = bacc.Bacc("TRN2", target_bir_lowering=False, debug=False)

    # All inputs are pre-laid-out host-side in SBUF partition-major form
    # [128, chunks, cols] so each tensor loads as ONE dma_start whose
    # descriptors are the full per-partition byte run (4-16KB) -- the DMA
    # engines are descriptor-rate limited (~100ns/descriptor/queue), so
    # descriptor size sets effective bandwidth.
    qT_d = nc.dram_tensor("qT", [128, QD // 128, LQ], bf16, kind="ExternalInput")
    kT_d = nc.dram_tensor("kT", [128, KD // 128, LK], bf16, kind="ExternalInput")
    vT_d = nc.dram_tensor("vT", [128, VD // 128, LK], bf16, kind="ExternalInput")
    wq_d = nc.dram_tensor("wq", [128, QD // 128, HH], bf16, kind="ExternalInput")
    wk_d = nc.dram_tensor("wk", [128, KD // 128, HH], bf16, kind="ExternalInput")
    wv_d = nc.dram_tensor("wv", [128, VD // 128, HH], bf16, kind="ExternalInput")
    wo_d = nc.dram_tensor("wo", [128, HH // 128, H], bf16, kind="ExternalInput")
    bq_d = nc.dram_tensor("bq", [128, 4], f32, kind="ExternalInput")
    bk_d = nc.dram_tensor("bk", [128, 4], f32, kind="ExternalInput")
    bv_d = nc.dram_tensor("bv", [1, HH], f32, kind="ExternalInput")
    out_d = nc.dram_tensor("out", [128, LQ // 128, H], f32,
                           kind="ExternalOutput")
    if debug:
        qp_dbg = nc.dram_tensor("qp_dbg", [128, 4 * LQ], bf16,
                                kind="ExternalOutput")
        kp_dbg = nc.dram_tensor("kp_dbg", [128, 4 * LK], bf16,
                                kind="ExternalOutput")
        es_dbg = nc.dram_tensor("es_dbg", [128, 4 * 1024], bf16,
                                kind="ExternalOutput")
        den_dbg = nc.dram_tensor("den_dbg", [8, 2 * 512], f32,
                                 kind="ExternalOutput")
        qt_dbg = nc.dram_tensor("qt_dbg", [128, LQ], bf16,
                                kind="ExternalOutput")
        wq_dbg = nc.dram_tensor("wq_dbg", [128, HH], bf16,
                                kind="ExternalOutput")

    QDC = QD // 128                       # 6 contraction chunks (q proj)
    KDC = KD // 128                       # 4 contraction chunks (k/v proj)
    MT = HH // 128                        # 4 m-tiles (head pairs)
    LKC = LK // 128                       # 16 key chunks
    QT = LQ // 128                        # 8 query tiles

    def dma_split(engine, dst, src, parts=4):
        """Split a [128, X] transfer into `parts` so descriptors spread
        across DMA queues (one partition row = one descriptor)."""
        step = 128 // parts
        for i in range(parts):
            engine.dma_start(dst[i * step:(i + 1) * step],
                             src[i * step:(i + 1) * step])

    with tile.TileContext(nc) as tc:
        with tc.tile_pool(name="persist", bufs=1) as per, \
             tc.tile_pool(name="es", bufs=22) as esp, \
             tc.tile_pool(name="bc", bufs=2) as bcp, \
             tc.tile_pool(name="rec", bufs=2) as rcp, \
             tc.tile_pool(name="osb", bufs=2) as osp, \
             tc.tile_pool(name="sps", bufs=2, space="PSUM") as spsp, \
             tc.tile_pool(name="aps", bufs=2, space="PSUM") as apsp:

            # ---- q-proj inputs first (they gate the first matmul) ----
            qt = per.tile([128, QDC, LQ], bf16)
            dma_split(nc.sync, qt[:], qT_d[:], 8)
            wq = per.tile([128, QDC, HH], bf16)
            dma_split(nc.sync, wq[:], wq_d[:], 8)
            bq = per.tile([128, 4], f32)
            nc.sync.dma_start(bq[:], bq_d[:])
            # k-proj inputs next in queue order
            kt = per.tile([128, KDC, LK], bf16)
            dma_split(nc.sync, kt[:], kT_d[:], 8)
            wk = per.tile([128, KDC, HH], bf16)
            dma_split(nc.sync, wk[:], wk_d[:], 4)
            bk = per.tile([128, 4], f32)
            nc.sync.dma_start(bk[:], bk_d[:])

            # ---- projection outputs ----
            qp = per.tile([128, MT, LQ], bf16)        # q^T  [512, 1024]
            kp = per.tile([128, MT, LK], bf16)        # k^T  [512, 2048]
            vp_t = [per.tile([128, NHH, HD + 1], bf16, name=f"vp{l}")
                    for l in range(LKC)]
            for l in range(LKC):
                nc.vector.memset(vp_t[l][:, :, HD:HD + 1], 1.0)
            attT = per.tile([128, MT, LQ], bf16)      # att^T [512, 1024]

            # interleave state ----------------------------------------
            S_seq = [(h, kc) for h in range(NHH) for kc in range(LKC)]
            NS = len(S_seq)
            LAG = 2
            ES_CAP = 20
            es_tiles = {}
            att_tiles = {}
            state = {"s": 0, "a": 0, "lkm_done": -1, "kn_done": -1,
                     "qm_done": -1}

            def emit_S():
                i = state["s"]
                h, kc = S_seq[i]
                hc, po = h // 2, 64 * (h % 2)
                sps = spsp.tile([128, 2, 512], f32, name="sps", tag="ps")
                for qb in range(2):
                    nc.tensor.matmul(sps[:, qb, :],
                                     kp[po:po + 64, hc,
                                        kc * 128:(kc + 1) * 128],
                                     qp[po:po + 64, hc,
                                        qb * 512:(qb + 1) * 512],
                                     start=True, stop=True)
                es = esp.tile([128, 2, 512], bf16, name="es", tag="es")
                nc.scalar.activation(es[:], sps[:], AF.Exp, scale=0.125)
                if debug and i < 4:
                    nc.sync.dma_start(
                        es_dbg[:, i * 1024:(i + 1) * 1024],
                        es[:].rearrange("p a b -> p (a b)"))
                es_tiles[i] = es
                state["s"] += 1

            def can_S():
                if state["s"] >= NS:
                    return False
                if state["s"] - state["a"] >= ES_CAP:
                    return False
                h, kc = S_seq[state["s"]]
                hc = h // 2
                # k-proj done through (m-tile hc, n-half covering kc)?
                need_kn = 2 * hc + (0 if kc < 8 else 1)
                return need_kn <= state["kn_done"] and hc <= state["qm_done"]

            def can_att():
                if state["a"] >= NS or state["a"] > state["s"] - LAG:
                    return False
                h, kc = S_seq[state["a"]]
                return kc <= state["lkm_done"]

            def emit_att():
                i = state["a"]
                h, kc = S_seq[i]
                hc, po = h // 2, 64 * (h % 2)
                if kc == 0:
                    att_tiles[h] = apsp.tile([128, 2, 512], f32, name="attps",
                                             tag="attps")
                aps = att_tiles[h]
                es = es_tiles.pop(i)
                for qb in range(2):
                    nc.tensor.matmul(aps[0:HD + 1, qb, :],
                                     vp_t[kc][:, h, :],
                                     es[:, qb, :],
                                     start=(kc == 0), stop=(kc == LKC - 1))
                if kc == LKC - 1:
                    # denom row lives at PSUM partition 64; custom-DVE recip
                    # and partition_broadcast only work at base partition 0,
                    # so hop it down with a cross-base DVE copy first.
                    den = rcp.tile([1, 2, 512], f32, name="den", tag="den")
                    nc.vector.tensor_copy(den[:], aps[HD:HD + 1, :, :])
                    rec = rcp.tile([1, 2, 512], f32, name="rec", tag="rec")
                    nc.vector.reciprocal_approx_fast(rec[:], den[:])
                    bcst = bcp.tile([64, 2, 512], f32, name="bcst", tag="bcst")
                    nc.gpsimd.partition_broadcast(bcst[:], rec[:])
                    if debug:
                        nc.sync.dma_start(
                            den_dbg[h:h + 1, :],
                            bcst[0:1, :, :].rearrange("p a b -> p (a b)"))
                    nc.vector.tensor_mul(
                        attT[po:po + 64, hc, :].rearrange(
                            "p (a b) -> p a b", a=2),
                        aps[0:HD, :, :], bcst[:])
                    del att_tiles[h]
                state["a"] += 1

            # phase QK: per head-pair m-tile: q-proj, then k-proj in n-halves
            for m in range(MT):
                ps = spsp.tile([128, 2, 512], f32)
                for qb in range(2):
                    for c in range(QDC):
                        nc.tensor.matmul(ps[:, qb, :],
                                         wq[:, c, m * 128:(m + 1) * 128],
                                         qt[:, c, qb * 512:(qb + 1) * 512],
                                         start=(c == 0), stop=(c == QDC - 1))
                for qb in range(2):
                    nc.vector.tensor_scalar_add(
                        qp[:, m, qb * 512:(qb + 1) * 512],
                        ps[:, qb, :], bq[:, m:m + 1])
                state["qm_done"] = m
                if m == 0:
                    # v-proj inputs: queue behind k-proj inputs
                    vt = per.tile([128, KDC, LK], bf16)
                    dma_split(nc.sync, vt[:], vT_d[:], 8)
                    wv = per.tile([128, KDC, HH], bf16)
                    dma_split(nc.sync, wv[:], wv_d[:], 4)
                    bv_bc = per.tile([128, HH], f32)
                    nc.gpsimd.dma_start(
                        out=bv_bc[:], in_=bv_d[0:1, :].to_broadcast([128, HH]))
                for np_ in range(2):
                    ps = spsp.tile([128, 2, 512], f32)
                    for j in range(2):
                        n = 2 * np_ + j
                        for c in range(KDC):
                            nc.tensor.matmul(ps[:, j, :],
                                             wk[:, c, m * 128:(m + 1) * 128],
                                             kt[:, c, n * 512:(n + 1) * 512],
                                             start=(c == 0),
                                             stop=(c == KDC - 1))
                    nc.vector.tensor_scalar_add(
                        kp[:, m, np_ * 1024:(np_ + 1) * 1024],
                        ps[:].rearrange("p a b -> p (a b)"), bk[:, m:m + 1])
                    state["kn_done"] = 2 * m + np_
                    for _ in range(4):
                        if can_S():
                            emit_S()

            if debug:
                nc.sync.dma_start(qp_dbg[:],
                                  qp[:].rearrange("p a b -> p (a b)"))
                nc.sync.dma_start(kp_dbg[:],
                                  kp[:].rearrange("p a b -> p (a b)"))
                nc.sync.dma_start(qt_dbg[:], qt[:, 0, :])
                nc.sync.dma_start(wq_dbg[:], wq[:, 0, :])

            # phase V: v-proj with S + att interleave
            for lkm in range(LKC):
                if lkm == 2:
                    wo = per.tile([128, MT, H], bf16)
                    dma_split(nc.sync, wo[:], wo_d[:], 4)
                ps = spsp.tile([128, 2, 512], f32)
                for c in range(KDC):
                    nc.tensor.matmul(ps[:, 0, :],
                                     vt[:, c, lkm * 128:(lkm + 1) * 128],
                                     wv[:, c, :],
                                     start=(c == 0), stop=(c == KDC - 1))
                nc.vector.tensor_add(
                    vp_t[lkm][:, :, 0:HD],
                    ps[:, 0, :].rearrange("p (h d) -> p h d", h=NHH),
                    bv_bc[:].rearrange("p (h d) -> p h d", h=NHH))
                state["lkm_done"] = lkm
                for _ in range(3):
                    if can_S():
                        emit_S()
                for _ in range(2):
                    if can_att():
                        emit_att()

            # drain attention
            while state["s"] < NS or state["a"] < NS:
                progressed = False
                if can_S():
                    emit_S()
                    progressed = True
                while can_att():
                    emit_att()
                    progressed = True
                if not progressed:
                    if state["a"] < NS and state["a"] < state["s"]:
                        emit_att()
                    elif state["s"] < NS:
                        emit_S()

            # ---- out-projection: out[qt] = attT[:, :, qt].T @ wo ----
            for qt in range(QT):
                ps = spsp.tile([128, 2, 512], f32)
                for n2 in range(2):
                    for c in range(MT):
                        nc.tensor.matmul(ps[:, n2, :],
                                         attT[:, c, qt * 128:(qt + 1) * 128],
                                         wo[:, c, n2 * 512:(n2 + 1) * 512],
                                         start=(c == 0), stop=(c == MT - 1))
                osb = osp.tile([128, H], f32)
                nc.vector.tensor_copy(
                    osb[:].rearrange("p (a b) -> p a b", a=2), ps[:])
                for i in range(2):
                    nc.sync.dma_start(
                        out_d[i * 64:(i + 1) * 64, qt, :],
                        osb[i * 64:(i + 1) * 64, :])

    nc.compile()
    return nc


def _get_nc():
    if "nc" not in _NC_CACHE:
        _NC_CACHE["nc"] = build_nc()
    return _NC_CACHE["nc"]


def make_in_maps(query, key, value, Wq, bq, Wk, bk, Wv, bv, Wo, bo):
    query = np.asarray(query, np.float32)
    key = np.asarray(key, np.float32)
    value = np.asarray(value, np.float32)
    Wq = np.asarray(Wq, np.float32)
    Wk = np.asarray(Wk, np.float32)
    Wv = np.asarray(Wv, np.float32)
    Wo = np.asarray(Wo, np.float32)
    bq = np.asarray(bq, np.float32)
    bk = np.asarray(bk, np.float32)
    bv = np.asarray(bv, np.float32)

    def pm(x):
        """[rows, cols] -> partition-major [128, rows//128, cols] bf16."""
        r, c = x.shape
        return np.ascontiguousarray(
            x.astype(_BF).reshape(r // 128, 128, c).transpose(1, 0, 2))

    per_batch = {}
    for b in range(B):
        per_batch[b] = {
            "qT": pm(query[b].T),
            "kT": pm(key[b].T),
            "vT": pm(value[b].T),
        }
    per_half = {}
    for hh in range(2):
        c0 = hh * HH
        per_half[hh] = {
            "wq": pm(Wq[:, c0:c0 + HH]),
            "wk": pm(Wk[:, c0:c0 + HH]),
            "wv": pm(Wv[:, c0:c0 + HH]),
            "wo": pm(Wo[c0:c0 + HH, :]),
            "bq": np.ascontiguousarray(bq[c0:c0 + HH].reshape(4, 128).T),
            "bk": np.ascontiguousarray(bk[c0:c0 + HH].reshape(4, 128).T),
            "bv": bv[c0:c0 + HH].reshape(1, HH).copy(),
        }
    in_maps = []
    for c in range(NCORES):
        b, hh = divmod(c, 2)
        m = dict(per_batch[b])
        m.update(per_half[hh])
        in_maps.append(m)
    return in_maps


def run(inputs, trace=False):
    from concourse.bass_utils import run_bass_kernel_spmd

    nc = _get_nc()
    in_maps = make_in_maps(**inputs)
    res = run_bass_kernel_spmd(nc, in_maps, list(range(NCORES)), trace=trace)
    bo = np.asarray(inputs["bo"], np.float32)
    out = np.empty((B, LQ, H), np.float32)
    for b in range(B):
        # device layout [128, LQ//128, H] partition-major -> [LQ, H]
        s = res.results[2 * b]["out"] + res.results[2 * b + 1]["out"]
        out[b] = s.transpose(1, 0, 2).reshape(LQ, H) + bo[None, :]
    return out, res


def kernel(**inputs):
    out, _ = run(inputs, trace=False)
    return out
